# revision 17
# baseline (speedup 1.0000x reference)
"""ConvBlock (proj -> depthwise causal conv1d -> silu, gated, out-proj) on 8 TRN2 NeuronCores.

Sharding: data-parallel over tokens (B*L = 8192 -> 1024 tokens/core) with a
3-token left halo per shard (zeros at batch starts), so the causal depthwise
conv needs no cross-core communication.

Per-core layout: activations are kept transposed [channels(partitions), tokens]
so the conv is a shifted access pattern along the free dim. All host-side
re-layouts (transposes, chunking) are done in numpy here. DMAs are batched
into few large transfers (HWDGE trigger overhead is ~625ns per dma_start).
"""
import numpy as np

import concourse.bacc as bacc
import concourse.mybir as mybir
import concourse.tile as tile
from concourse.bass_utils import run_bass_kernel_spmd

F32 = mybir.dt.float32
F32R = mybir.dt.float32r
BF16 = mybir.dt.bfloat16
AF = mybir.ActivationFunctionType

B, L, D, E, DC = 2, 4096, 1024, 2048, 4
NCORES = 8
T = B * L // NCORES          # 1024 tokens per core
H = DC - 1                   # 3 halo tokens
TH = T + H + 1               # 1028 (+1 zero pad col: fp32r needs even matmul dims)
ED = E // 128                # 16 e-chunks
KD = D // 128                # 8 d-chunks (contraction for proj/gate)
DM = D // 128                # 8 output-row chunks
NT = 512                     # matmul moving-dim tile (PSUM bank limit, f32)

# dtype knobs
MM_DT = BF16                 # proj/gate matmul operand dtype
EW_DT = BF16                 # elementwise dtype for val/sv/sg staging
H_DT = BF16                  # h (= out-matmul rhs) dtype; wot matches


def _build_nc(reps=1):
    nc = bacc.Bacc("TRN2", target_bir_lowering=False, debug=False,
                   num_devices=NCORES)

    xt = nc.dram_tensor("xt", [KD, 128, TH], MM_DT, kind="ExternalInput").ap()
    # wpg[e] = [128, 2048]: cols 0:1024 proj lhsT k-tiles, cols 1024:2048 gate
    wpg = nc.dram_tensor("wpg", [ED, 128, 2 * KD * 128], MM_DT,
                         kind="ExternalInput").ap()
    # wot[j] = [128, ED*128]: out-proj lhsT tiles for row-chunk j, all e side by side
    wot = nc.dram_tensor("wot", [DM, 128, ED * 128], H_DT,
                         kind="ExternalInput").ap()
    wcv = nc.dram_tensor("wcv", [128, ED * DC], F32, kind="ExternalInput").ap()
    bcv = nc.dram_tensor("bcv", [128, ED], F32, kind="ExternalInput").ap()
    yt = nc.dram_tensor("yt", [DM, 128, T], F32, kind="ExternalOutput").ap()

    with tile.TileContext(nc) as tc:
        with tc.tile_pool(name="xp", bufs=1) as xp, \
             tc.tile_pool(name="cp", bufs=1) as cp, \
             tc.tile_pool(name="hp", bufs=1) as hp, \
             tc.tile_pool(name="wp", bufs=2) as wp, \
             tc.tile_pool(name="vp", bufs=2) as vp, \
             tc.tile_pool(name="tp", bufs=2) as tp, \
             tc.tile_pool(name="wo", bufs=3) as wopool, \
             tc.tile_pool(name="yp", bufs=2) as yp, \
             tc.tile_pool(name="ps", bufs=1, space="PSUM") as ps:

            # first proj weight tile + first x chunk lead the DMA queue so the
            # PE can start at ~5us instead of waiting for the full x transfer
            wpg0 = wp.tile([128, 2 * KD * 128], MM_DT, name="wpg_t", tag="wpg")
            nc.sync.dma_start(wpg0[:], wpg[0])
            xt_sb = []
            for k in range(KD):
                x_k = xp.tile([128, TH], MM_DT, name=f"x_{k}")
                nc.sync.dma_start(x_k[:], xt[k])
                xt_sb.append(x_k)
            wcv_sb = cp.tile([128, ED * DC], F32, name="wcv_sb")
            nc.sync.dma_start(wcv_sb[:], wcv[:])
            bcv_sb = cp.tile([128, ED], F32, name="bcv_sb")
            nc.sync.dma_start(bcv_sb[:], bcv[:])
            # warm the ACT Silu table while initial DMAs are in flight
            silu_warm = cp.tile([128, 2], F32, name="silu_warm")
            nc.gpsimd.memset(silu_warm[:], 0.0)
            nc.scalar.activation(silu_warm[:], silu_warm[:], AF.Silu)

            h_all = hp.tile([128, ED * T], H_DT, name="h_all")

            val_cols = [(0, NT), (NT, NT), (2 * NT, TH - 2 * NT)]

            def phase_a(e, wpg_t):
                pv = []
                for n, (c0, w) in enumerate(val_cols):
                    p = ps.tile([128, NT], F32, name=f"pv{n}", tag=f"pv{n}",
                                bufs=(2 if n == 0 else 1))
                    for k in range(KD):
                        nc.tensor.matmul(
                            p[:, :w], wpg_t[:, k * 128:(k + 1) * 128],
                            xt_sb[k][:, c0:c0 + w],
                            start=(k == 0), stop=(k == KD - 1))
                    pv.append(p)
                pg = []
                for n in range(2):
                    c0 = H + n * NT
                    p = ps.tile([128, NT], F32, name=f"pg{n}", tag=f"pg{n}")
                    for k in range(KD):
                        nc.tensor.matmul(
                            p[:], wpg_t[:, (KD + k) * 128:(KD + k + 1) * 128],
                            xt_sb[k][:, c0:c0 + NT],
                            start=(k == 0), stop=(k == KD - 1))
                    pg.append(p)

                # stage val (PSUM -> SBUF) on ACT
                val_sb = vp.tile([128, TH], EW_DT, name="val_sb", tag="val")
                for n, (c0, w) in enumerate(val_cols):
                    nc.scalar.copy(val_sb[:, c0:c0 + w], pv[n][:, :w])
                # silu(gate) (PSUM -> SBUF) on ACT
                sg = tp.tile([128, T], EW_DT, name="sg", tag="sg")
                for n in range(2):
                    nc.scalar.activation(sg[:, n * NT:(n + 1) * NT], pg[n][:],
                                         AF.Silu)

                # depthwise causal conv: acc = sum_k wc[:,k] * val[:, k:k+T]
                acc = tp.tile([128, T], EW_DT, name="acc", tag="acc")
                nc.vector.tensor_scalar_mul(acc[:], val_sb[:, 0:T],
                                            wcv_sb[:, e * DC:e * DC + 1])
                for kk in range(1, DC):
                    m = tp.tile([128, T], EW_DT, name="m", tag="m")
                    nc.vector.tensor_scalar_mul(
                        m[:], val_sb[:, kk:kk + T],
                        wcv_sb[:, e * DC + kk:e * DC + kk + 1])
                    nc.vector.tensor_add(acc[:], acc[:], m[:])
                # silu(conv + b_conv) on ACT
                sv = tp.tile([128, T], EW_DT, name="sv", tag="sv")
                nc.scalar.activation(sv[:], acc[:], AF.Silu,
                                     bias=bcv_sb[:, e:e + 1])
                # h = silu(v) * silu(gate)
                nc.vector.tensor_mul(h_all[:, e * T:(e + 1) * T], sv[:], sg[:])

            def phase_b(j):
                wo_t = wopool.tile([128, ED * 128], H_DT, name="wo_t", tag="wo")
                nc.sync.dma_start(wo_t[:], wot[j])
                y_out = yp.tile([128, T], F32, name="y_out", tag="yo")
                for n in range(2):
                    yb = ps.tile([128, NT], F32, name="yb", tag="yb", bufs=2)
                    for e in range(ED):
                        nc.tensor.matmul(
                            yb[:], wo_t[:, e * 128:(e + 1) * 128],
                            h_all[:, e * T + n * NT: e * T + (n + 1) * NT],
                            start=(e == 0), stop=(e == ED - 1))
                    nc.scalar.copy(y_out[:, n * NT:(n + 1) * NT], yb[:])
                nc.sync.dma_start(yt[j], y_out[:])

            for rep in range(reps):
                for e in range(ED):
                    if rep == 0 and e == 0:
                        wpg_t = wpg0
                    else:
                        wpg_t = wp.tile([128, 2 * KD * 128], MM_DT,
                                        name="wpg_t", tag="wpg")
                        nc.sync.dma_start(wpg_t[:], wpg[e])
                    phase_a(e, wpg_t)
                for j in range(DM):
                    phase_b(j)

    nc.compile()
    return nc


_NC_CACHE = {}


def _get_nc():
    if "nc" not in _NC_CACHE:
        _NC_CACHE["nc"] = _build_nc()
    return _NC_CACHE["nc"]


def make_in_maps(x, W_proj, W_gate, W_conv, b_conv, W_out):
    """Host-side sharding + re-layout into per-core input dicts."""
    mm_np = mybir.dt.np(MM_DT)
    h_np = mybir.dt.np(H_DT)
    xf = np.ascontiguousarray(np.asarray(x, dtype=np.float32).reshape(B * L, D))
    # lhsT tile layouts: wpt[e, kp, k*128+m] = W_proj[e*128+m, k*128+kp]
    wpt = (np.asarray(W_proj, np.float32).reshape(ED, 128, KD, 128)
           .transpose(0, 3, 2, 1).reshape(ED, 128, KD * 128))
    wgt = (np.asarray(W_gate, np.float32).reshape(ED, 128, KD, 128)
           .transpose(0, 3, 2, 1).reshape(ED, 128, KD * 128))
    wpg = np.ascontiguousarray(
        np.concatenate([wpt, wgt], axis=2).astype(mm_np))
    # wot[j, p, e*128+m] = W_out[j*128+m, e*128+p]
    wot = np.ascontiguousarray(
        np.asarray(W_out, np.float32).reshape(DM, 128, ED, 128)
        .transpose(0, 3, 2, 1).reshape(DM, 128, ED * 128).astype(h_np))
    # wcv[p, e*DC+k] = W_conv[e*128+p, 0, k]; bcv[p, e] = b_conv[e*128+p]
    wcv = np.ascontiguousarray(
        np.asarray(W_conv, np.float32).reshape(ED, 128, DC)
        .transpose(1, 0, 2).reshape(128, ED * DC))
    bcv = np.ascontiguousarray(
        np.asarray(b_conv, np.float32).reshape(ED, 128).T)

    in_maps = []
    for c in range(NCORES):
        s = c * T
        hx = np.zeros((TH, D), dtype=np.float32)
        if s % L != 0:
            hx[0:H] = xf[s - H:s]
        hx[H:H + T] = xf[s:s + T]
        xt = np.ascontiguousarray(hx.T.reshape(KD, 128, TH).astype(mm_np))
        in_maps.append({"xt": xt, "wpg": wpg, "wot": wot,
                       "wcv": wcv, "bcv": bcv})
    return in_maps


def assemble_output(results):
    out = np.empty((B * L, D), dtype=np.float32)
    for c in range(NCORES):
        yt = results[c]["yt"]              # [DM, 128, T]
        out[c * T:(c + 1) * T] = yt.reshape(D, T).T
    return out.reshape(B, L, D)


def kernel(x, W_proj, W_gate, W_conv, b_conv, W_out):
    nc = _get_nc()
    in_maps = make_in_maps(x, W_proj, W_gate, W_conv, b_conv, W_out)
    res = run_bass_kernel_spmd(nc, in_maps, core_ids=list(range(NCORES)))
    return assemble_output(res.results)


# revision 18
# speedup vs baseline: 1.3483x; 1.3483x over previous
"""ConvBlock (proj -> depthwise causal conv1d -> silu, gated, out-proj) on 8 TRN2 NeuronCores.

Sharding: data-parallel over tokens (B*L = 8192 -> 1024 tokens/core) with a
3-token left halo per shard (zeros at batch starts), so the causal depthwise
conv needs no cross-core communication.

Per-core layout: activations are kept transposed [channels(partitions), tokens]
so the conv is a shifted access pattern along the free dim. All host-side
re-layouts (transposes, chunking) are done in numpy here. DMAs are batched
into few large transfers (HWDGE trigger overhead is ~625ns per dma_start).
"""
import numpy as np

import concourse.bacc as bacc
import concourse.mybir as mybir
import concourse.tile as tile
from concourse.bass_utils import run_bass_kernel_spmd

F32 = mybir.dt.float32
F32R = mybir.dt.float32r
BF16 = mybir.dt.bfloat16
AF = mybir.ActivationFunctionType

B, L, D, E, DC = 2, 4096, 1024, 2048, 4
NCORES = 8
T = B * L // NCORES          # 1024 tokens per core
H = DC - 1                   # 3 halo tokens
TH = T + H + 1               # 1028 (+1 zero pad col: fp32r needs even matmul dims)
ED = E // 128                # 16 e-chunks
KD = D // 128                # 8 d-chunks (contraction for proj/gate)
DM = D // 128                # 8 output-row chunks
NT = 512                     # matmul moving-dim tile (PSUM bank limit, f32)

# dtype knobs
MM_DT = BF16                 # proj/gate matmul operand dtype
EW_DT = BF16                 # elementwise dtype for val/sv/sg staging
H_DT = BF16                  # h (= out-matmul rhs) dtype; wot matches


def _build_nc(reps=1):
    nc = bacc.Bacc("TRN2", target_bir_lowering=False, debug=False,
                   num_devices=NCORES)

    xt = nc.dram_tensor("xt", [KD, 128, TH], MM_DT, kind="ExternalInput").ap()
    # wpg[e] = [128, 2048]: cols 0:1024 proj lhsT k-tiles, cols 1024:2048 gate
    wpg = nc.dram_tensor("wpg", [ED, 128, 2 * KD * 128], MM_DT,
                         kind="ExternalInput").ap()
    # wot[j] = [128, ED*128]: out-proj lhsT tiles for row-chunk j, all e side by side
    wot = nc.dram_tensor("wot", [DM, 128, ED * 128], H_DT,
                         kind="ExternalInput").ap()
    wcv = nc.dram_tensor("wcv", [128, ED * DC], F32, kind="ExternalInput").ap()
    bcv = nc.dram_tensor("bcv", [128, ED], F32, kind="ExternalInput").ap()
    yt = nc.dram_tensor("yt", [DM, 128, T], F32, kind="ExternalOutput").ap()

    with tile.TileContext(nc) as tc:
        with tc.tile_pool(name="xp", bufs=1) as xp, \
             tc.tile_pool(name="cp", bufs=1) as cp, \
             tc.tile_pool(name="hp", bufs=1) as hp, \
             tc.tile_pool(name="wp", bufs=3) as wp, \
             tc.tile_pool(name="vp", bufs=3) as vp, \
             tc.tile_pool(name="tp", bufs=2) as tp, \
             tc.tile_pool(name="wo", bufs=5) as wopool, \
             tc.tile_pool(name="yp", bufs=2) as yp, \
             tc.tile_pool(name="ps", bufs=1, space="PSUM") as ps:

            # first proj weight tile + first x chunk lead the DMA queue so the
            # PE can start at ~5us instead of waiting for the full x transfer
            wpg0 = wp.tile([128, 2 * KD * 128], MM_DT, name="wpg_t", tag="wpg")
            nc.sync.dma_start(wpg0[:], wpg[0])
            xt_sb = []
            for k in range(KD):
                x_k = xp.tile([128, TH], MM_DT, name=f"x_{k}")
                nc.sync.dma_start(x_k[:], xt[k])
                xt_sb.append(x_k)
            wcv_sb = cp.tile([128, ED * DC], F32, name="wcv_sb")
            nc.sync.dma_start(wcv_sb[:], wcv[:])
            bcv_sb = cp.tile([128, ED], F32, name="bcv_sb")
            nc.sync.dma_start(bcv_sb[:], bcv[:])
            # warm the ACT Silu table while initial DMAs are in flight
            silu_warm = cp.tile([128, 2], F32, name="silu_warm")
            nc.gpsimd.memset(silu_warm[:], 0.0)
            nc.scalar.activation(silu_warm[:], silu_warm[:], AF.Silu)

            h_all = hp.tile([128, ED * T], H_DT, name="h_all")

            val_cols = [(0, NT), (NT, NT), (2 * NT, TH - 2 * NT)]

            def phase_a(e, wpg_t):
                pv = []
                for n, (c0, w) in enumerate(val_cols):
                    p = ps.tile([128, NT], F32, name=f"pv{n}", tag=f"pv{n}",
                                bufs=(2 if n == 0 else 1))
                    for k in range(KD):
                        nc.tensor.matmul(
                            p[:, :w], wpg_t[:, k * 128:(k + 1) * 128],
                            xt_sb[k][:, c0:c0 + w],
                            start=(k == 0), stop=(k == KD - 1))
                    pv.append(p)
                pg = []
                for n in range(2):
                    c0 = H + n * NT
                    p = ps.tile([128, NT], F32, name=f"pg{n}", tag=f"pg{n}")
                    for k in range(KD):
                        nc.tensor.matmul(
                            p[:], wpg_t[:, (KD + k) * 128:(KD + k + 1) * 128],
                            xt_sb[k][:, c0:c0 + NT],
                            start=(k == 0), stop=(k == KD - 1))
                    pg.append(p)

                # stage val (PSUM -> SBUF) on ACT
                val_sb = vp.tile([128, TH], EW_DT, name="val_sb", tag="val")
                for n, (c0, w) in enumerate(val_cols):
                    nc.scalar.copy(val_sb[:, c0:c0 + w], pv[n][:, :w])
                # silu(gate) (PSUM -> SBUF) on ACT
                sg = tp.tile([128, T], EW_DT, name="sg", tag="sg")
                for n in range(2):
                    nc.scalar.activation(sg[:, n * NT:(n + 1) * NT], pg[n][:],
                                         AF.Silu)

                # depthwise causal conv: acc = sum_k wc[:,k] * val[:, k:k+T]
                acc = tp.tile([128, T], EW_DT, name="acc", tag="acc")
                nc.vector.tensor_scalar_mul(acc[:], val_sb[:, 0:T],
                                            wcv_sb[:, e * DC:e * DC + 1])
                for kk in range(1, DC):
                    m = tp.tile([128, T], EW_DT, name="m", tag="m")
                    nc.vector.tensor_scalar_mul(
                        m[:], val_sb[:, kk:kk + T],
                        wcv_sb[:, e * DC + kk:e * DC + kk + 1])
                    nc.vector.tensor_add(acc[:], acc[:], m[:])
                # silu(conv + b_conv) on ACT
                sv = tp.tile([128, T], EW_DT, name="sv", tag="sv")
                nc.scalar.activation(sv[:], acc[:], AF.Silu,
                                     bias=bcv_sb[:, e:e + 1])
                # h = silu(v) * silu(gate)
                nc.vector.tensor_mul(h_all[:, e * T:(e + 1) * T], sv[:], sg[:])

            def phase_b(j):
                wo_t = wopool.tile([128, ED * 128], H_DT, name="wo_t", tag="wo")
                nc.sync.dma_start(wo_t[:], wot[j])
                y_out = yp.tile([128, T], F32, name="y_out", tag="yo")
                for n in range(2):
                    yb = ps.tile([128, NT], F32, name="yb", tag="yb", bufs=2)
                    for e in range(ED):
                        nc.tensor.matmul(
                            yb[:], wo_t[:, e * 128:(e + 1) * 128],
                            h_all[:, e * T + n * NT: e * T + (n + 1) * NT],
                            start=(e == 0), stop=(e == ED - 1))
                    nc.scalar.copy(y_out[:, n * NT:(n + 1) * NT], yb[:])
                nc.sync.dma_start(yt[j], y_out[:])

            for rep in range(reps):
                for e in range(ED):
                    if rep == 0 and e == 0:
                        wpg_t = wpg0
                    else:
                        wpg_t = wp.tile([128, 2 * KD * 128], MM_DT,
                                        name="wpg_t", tag="wpg")
                        nc.sync.dma_start(wpg_t[:], wpg[e])
                    phase_a(e, wpg_t)
                for j in range(DM):
                    phase_b(j)

    nc.compile()
    return nc


_NC_CACHE = {}


def _get_nc():
    if "nc" not in _NC_CACHE:
        _NC_CACHE["nc"] = _build_nc()
    return _NC_CACHE["nc"]


def make_in_maps(x, W_proj, W_gate, W_conv, b_conv, W_out):
    """Host-side sharding + re-layout into per-core input dicts."""
    mm_np = mybir.dt.np(MM_DT)
    h_np = mybir.dt.np(H_DT)
    xf = np.ascontiguousarray(np.asarray(x, dtype=np.float32).reshape(B * L, D))
    # lhsT tile layouts: wpt[e, kp, k*128+m] = W_proj[e*128+m, k*128+kp]
    wpt = (np.asarray(W_proj, np.float32).reshape(ED, 128, KD, 128)
           .transpose(0, 3, 2, 1).reshape(ED, 128, KD * 128))
    wgt = (np.asarray(W_gate, np.float32).reshape(ED, 128, KD, 128)
           .transpose(0, 3, 2, 1).reshape(ED, 128, KD * 128))
    wpg = np.ascontiguousarray(
        np.concatenate([wpt, wgt], axis=2).astype(mm_np))
    # wot[j, p, e*128+m] = W_out[j*128+m, e*128+p]
    wot = np.ascontiguousarray(
        np.asarray(W_out, np.float32).reshape(DM, 128, ED, 128)
        .transpose(0, 3, 2, 1).reshape(DM, 128, ED * 128).astype(h_np))
    # wcv[p, e*DC+k] = W_conv[e*128+p, 0, k]; bcv[p, e] = b_conv[e*128+p]
    wcv = np.ascontiguousarray(
        np.asarray(W_conv, np.float32).reshape(ED, 128, DC)
        .transpose(1, 0, 2).reshape(128, ED * DC))
    bcv = np.ascontiguousarray(
        np.asarray(b_conv, np.float32).reshape(ED, 128).T)

    in_maps = []
    for c in range(NCORES):
        s = c * T
        hx = np.zeros((TH, D), dtype=np.float32)
        if s % L != 0:
            hx[0:H] = xf[s - H:s]
        hx[H:H + T] = xf[s:s + T]
        xt = np.ascontiguousarray(hx.T.reshape(KD, 128, TH).astype(mm_np))
        in_maps.append({"xt": xt, "wpg": wpg, "wot": wot,
                       "wcv": wcv, "bcv": bcv})
    return in_maps


def assemble_output(results):
    out = np.empty((B * L, D), dtype=np.float32)
    for c in range(NCORES):
        yt = results[c]["yt"]              # [DM, 128, T]
        out[c * T:(c + 1) * T] = yt.reshape(D, T).T
    return out.reshape(B, L, D)


def kernel(x, W_proj, W_gate, W_conv, b_conv, W_out):
    nc = _get_nc()
    in_maps = make_in_maps(x, W_proj, W_gate, W_conv, b_conv, W_out)
    res = run_bass_kernel_spmd(nc, in_maps, core_ids=list(range(NCORES)))
    return assemble_output(res.results)
